# revision 37
# baseline (speedup 1.0000x reference)
"""Trainium2 Bass kernel for nn_MultiHeadAttention (B=2, S=2048, D=2048, H=16, HD=128).

Sharding: tensor-parallel across heads. Each of the 8 cores owns 2 heads:
  - QKV projection for its heads (bf16 matmuls, fp32 PSUM accumulation);
    weight/x DMAs split into parts so the first chains start early
  - RoPE on DVE with host-precomputed bf16 cos / sign-folded sin tables
  - causal attention in transposed layout scores^T[k, q], depth-3 software
    pipelined (scores/exp issued three k-tiles ahead of attn@V so the PE
    rarely waits on the activation engine):
      exp via ScalarE (scale = HD^-0.5 folded in), diagonal tiles computed
      only on their valid [j0:512] q-subrange with a [128,128] triangle mask
      on DVE, attn @ V accumulated as out^T[d, q], softmax denominator via
      ones-vector matmuls on PE with adjacent full tiles pair-summed on DVE
      first (halves the rowsum matmul count)
  - normalization split in two: reciprocal_approx_fast issued on DVE the
    moment a chunk's rowsum closes, broadcast (PE outer product) + multiply
    issued one chunk later so the PE never stalls on the reciprocal
  - per-head AllToAll (4 collectives) redistributes head-outputs into
    sequence-slices; head 0's collective overlaps head 1's attention, and
    batch 1's last collective overlaps batch 0's output projection
  - output projection W_o (resident in SBUF, loaded once in the background),
    accumulating even heads first since their A2A lands a collective earlier
Host gathers by concatenating the 8 row-slices.
"""

import numpy as np

B = 2
H = 16
HD = 128
N_CORES = 8
HEADS_PER_CORE = H // N_CORES


# ---------------------------------------------------------------- device code
def build_nc(S=2048, D=2048, n_cores=N_CORES):
    from contextlib import ExitStack

    import concourse.mybir as mybir
    import concourse.tile as tile
    from concourse import bacc

    f32 = mybir.dt.float32
    bf16 = mybir.dt.bfloat16
    Exp = mybir.ActivationFunctionType.Exp

    KT = D // 128            # contraction tiles for projections
    S2 = B * S               # total rows of x
    NCH = S // 512           # 512-wide q-chunks per batch
    SLICE = S2 // n_cores    # output rows per core
    ECH = D // 512           # 512-wide e-chunks of the output dim
    HSC = HEADS_PER_CORE
    SL8 = S // n_cores
    hd_scale = float(HD) ** -0.5

    nc = bacc.Bacc("TRN2", target_bir_lowering=False, debug=False,
                   num_devices=n_cores)

    xt = nc.dram_tensor("xt", [D, S2], bf16, kind="ExternalInput").ap()
    wqk = nc.dram_tensor("wqk", [D, 2 * HSC * 128], bf16, kind="ExternalInput").ap()
    wv = nc.dram_tensor("wv", [D, HSC * 128], bf16, kind="ExternalInput").ap()
    wo = nc.dram_tensor("wo", [H * HD, D], bf16, kind="ExternalInput").ap()
    cost = nc.dram_tensor("cost", [128, S], bf16, kind="ExternalInput").ap()
    sins = nc.dram_tensor("sins", [128, S], bf16, kind="ExternalInput").ap()
    trit = nc.dram_tensor("trit", [128, 128], bf16, kind="ExternalInput").ap()
    ones = nc.dram_tensor("ones", [128, 1], bf16, kind="ExternalInput").ap()
    onesr = nc.dram_tensor("onesr", [1, 128], bf16, kind="ExternalInput").ap()
    out = nc.dram_tensor("out", [SLICE, D], f32, kind="ExternalOutput").ap()

    xt_rs = xt.rearrange("(kt p) s -> p kt s", p=128)
    wqk_rs = wqk.rearrange("(kt p) m -> p kt m", p=128)
    wv_rs = wv.rearrange("(kt p) m -> p kt m", p=128)
    wo_rs = wo.rearrange("(ht p) e -> p ht e", p=128)

    with tile.TileContext(nc) as tc, ExitStack() as ctx:
        const = ctx.enter_context(tc.tile_pool(name="const", bufs=1))
        stream = ctx.enter_context(tc.tile_pool(name="stream", bufs=2))
        qkvp = ctx.enter_context(tc.tile_pool(name="qkvp", bufs=1))
        expp = ctx.enter_context(tc.tile_pool(name="expp", bufs=5))
        tmp = ctx.enter_context(tc.tile_pool(name="tmp", bufs=2))
        psA = ctx.enter_context(tc.tile_pool(name="psA", bufs=3, space="PSUM"))
        psB = ctx.enter_context(tc.tile_pool(name="psB", bufs=2, space="PSUM"))
        psC = ctx.enter_context(tc.tile_pool(name="psC", bufs=1, space="PSUM"))
        psD = ctx.enter_context(tc.tile_pool(name="psD", bufs=2, space="PSUM"))
        dram = ctx.enter_context(tc.tile_pool(name="dram", bufs=1, space="DRAM"))

        # resident constants, ordered so the first matmuls can start early:
        # wv (1MB) + the first x s-piece feed the V projection immediately.
        wv_sb = const.tile([128, KT, HSC * 128], bf16)
        xt_first = stream.tile([128, KT, 512], bf16, tag="stream", name="xt_b0e0")
        wqk_sb = const.tile([128, KT, 2 * HSC * 128], bf16)
        cos_sb = const.tile([128, S], bf16)
        sins_sb = const.tile([128, S], bf16)
        tri_sb = const.tile([128, 128], bf16)
        ones_sb = const.tile([128, 1], bf16)
        onesr_sb = const.tile([1, 128], bf16)
        for p4 in range(4):
            nc.sync.dma_start(wv_sb[:, p4 * 4:(p4 + 1) * 4, :],
                              wv_rs[:, p4 * 4:(p4 + 1) * 4, :])
        for sp in range(4):
            nc.sync.dma_start(xt_first[:, :, sp * 128:(sp + 1) * 128],
                              xt_rs[:, :, sp * 128:(sp + 1) * 128])
        for p8 in range(8):
            nc.sync.dma_start(wqk_sb[:, p8 * 2:(p8 + 1) * 2, :],
                              wqk_rs[:, p8 * 2:(p8 + 1) * 2, :])
        nc.sync.dma_start(cos_sb[:], cost[:])
        nc.sync.dma_start(sins_sb[:], sins[:])
        nc.sync.dma_start(tri_sb[:], trit[:])
        nc.sync.dma_start(ones_sb[:], ones[:])
        nc.sync.dma_start(onesr_sb[:], onesr[:])

        # W_o resident; its 4 DMA parts are issued during batch-0 attention
        wo_sb = const.tile([128, H, D], bf16)

        def load_wo_part(j):
            nc.sync.dma_start(wo_sb[:, 4 * j:4 * (j + 1), :],
                              wo_rs[:, 4 * j:4 * (j + 1), :])

        pend = []        # deferred per-chunk normalization applications
        pend_rs = []     # deferred per-chunk rowsum + reciprocal stages
        rcbs = []        # reciprocal tiles handed from rs_stage to norm

        a2a_outs = {}    # (b, h) -> DRAM tile
        atn_sb = {}      # (b, h) -> SBUF tile [128, n_cores, SL8]

        def load_atn(b, h, split=1):
            t = stream.tile([128, n_cores, SL8], bf16, tag=f"atn{h}",
                            name=f"atn_{b}{h}")
            src = a2a_outs[b, h].rearrange("r p s -> p r s")
            w = SL8 // split
            for sp in range(split):
                nc.sync.dma_start(t[:, :, sp * w:(sp + 1) * w],
                                  src[:, :, sp * w:(sp + 1) * w])
            atn_sb[b, h] = t

        for b in range(B):
            # normalized attention output, transposed: [d, head, s] (bf16);
            # same tag so batch 1 reuses batch 0's buffer after its A2A reads
            attnT_sb = qkvp.tile([128, HSC, S], bf16, tag="attnT",
                                 name=f"attnT_{b}")

            # -------- QKV projection for batch b (heads of this core) --------
            q_sb = [qkvp.tile([128, S], bf16, tag=f"q{h}", name=f"q{h}_{b}")
                    for h in range(HSC)]
            k_sb = [qkvp.tile([128, S], bf16, tag=f"k{h}", name=f"k{h}_{b}")
                    for h in range(HSC)]
            v_sb = qkvp.tile([128, S // 128, HSC * 128], bf16, tag="v")

            for e4 in range(S // 512):
                s0 = e4 * 512
                if b == 0 and e4 == 0:
                    xt_sb = xt_first
                else:
                    xt_sb = stream.tile([128, KT, 512], bf16, tag="stream",
                                        name=f"xt_{b}{e4}")
                    nc.sync.dma_start(
                        xt_sb[:], xt_rs[:, :, b * S + s0:b * S + s0 + 512])

                # v tiles first (only needs wv + x): natural [s, c] layout
                for sv in range(4):
                    psv = psC.tile([128, HSC * 128], f32, tag="psC")
                    for kt in range(KT):
                        nc.tensor.matmul(
                            psv,
                            xt_sb[:, kt, sv * 128:(sv + 1) * 128],
                            wv_sb[:, kt, :],
                            start=(kt == 0), stop=(kt == KT - 1),
                        )
                    nc.scalar.copy(v_sb[:, e4 * 4 + sv, :], psv[:])

                # q/k tiles: out^T layout [c, s], N=512
                for ct in range(2 * HSC):
                    ps = psA.tile([128, 512], f32, tag="psA", name="ps_qk")
                    for kt in range(KT):
                        nc.tensor.matmul(
                            ps,
                            wqk_sb[:, kt, ct * 128:(ct + 1) * 128],
                            xt_sb[:, kt, :],
                            start=(kt == 0), stop=(kt == KT - 1),
                        )
                    dst = q_sb[ct] if ct < HSC else k_sb[ct - HSC]
                    sl = slice(s0, s0 + 512)
                    # RoPE: dst = ps*cos + swap_half(ps)*sign_sin
                    t1 = tmp.tile([128, 512], f32, tag="ropetmp")
                    nc.vector.tensor_mul(t1[:], ps, cos_sb[:, sl])
                    t2 = tmp.tile([128, 512], f32, tag="ropetmp2")
                    nc.vector.tensor_mul(t2[0:64, :], ps[64:128, :], sins_sb[0:64, sl])
                    nc.vector.tensor_mul(t2[64:128, :], ps[0:64, :], sins_sb[64:128, sl])
                    nc.vector.tensor_add(dst[:, sl], t1[:], t2[:])



            # -------- causal attention for batch b --------
            def stage_a2a(h):
                a_in = dram.tile([n_cores, 128, SL8], bf16,
                                 name=f"a2a_in_{b}{h}")
                a_out = dram.tile([n_cores, 128, SL8], bf16,
                                  name=f"a2a_out_{b}{h}")
                for j in range(n_cores):
                    nc.sync.dma_start(
                        a_in[j],
                        attnT_sb[:, h, j * SL8:(j + 1) * SL8],
                    )
                nc.gpsimd.collective_compute(
                    "AllToAll",
                    mybir.AluOpType.bypass,
                    replica_groups=[list(range(n_cores))],
                    ins=[a_in.opt()],
                    outs=[a_out.opt()],
                )
                a2a_outs[b, h] = a_out

            for h in range(HSC):
                if h == 1:
                    # head 0 is final: ship it while head 1 computes
                    while pend_rs:
                        pend_rs.pop(0)()
                    while pend:
                        pend.pop(0)()
                    stage_a2a(0)
                    if b == 1:
                        # batch-0 A2A gathers, off the projection critical path
                        load_atn(0, 0)
                        load_atn(0, 1)
                qh, kh = q_sb[h], k_sb[h]
                for c in range(NCH):
                    nkt = 4 * c + 4
                    nfull = 4 * c           # full-width tiles (paired rowsum)
                    nrs = nfull // 2 + 4    # rowsum matmuls in this chunk
                    av = psB.tile([128, 512], f32, tag="psB")
                    rs = psD.tile([1, 512], f32, tag="psD")
                    exs = {}
                    rsi = [0]

                    def produce(kt, c=c, qh=qh, exs=exs, nfull=nfull):
                        # diagonal tiles: only the valid q-subrange [lo:512)
                        lo = 128 * kt - 512 * c if kt >= 4 * c else 0
                        sc = psA.tile([128, 512], f32, tag="psA", name="sc")
                        nc.tensor.matmul(
                            sc[:, lo:],
                            kh[:, kt * 128:(kt + 1) * 128],
                            qh[:, c * 512 + lo:(c + 1) * 512],
                        )
                        ex = expp.tile([128, 512], bf16)
                        nc.scalar.activation(ex[:, lo:], sc[:, lo:], Exp,
                                             scale=hd_scale)
                        if kt >= 4 * c:  # triangle mask on the 128-wide band
                            nc.vector.tensor_mul(ex[:, lo:lo + 128],
                                                 ex[:, lo:lo + 128], tri_sb[:])
                        exs[kt] = (ex, lo)
                        if kt < nfull and kt % 2 == 1:
                            # pair-sum adjacent full tiles on DVE so a single
                            # rowsum matmul covers both
                            pr = tmp.tile([128, 512], bf16, tag="expair",
                                          bufs=2)
                            nc.vector.tensor_add(pr[:], exs[kt - 1][0][:],
                                                 ex[:])
                            exs['p', kt] = pr

                    def consume(kt, c=c, h=h, av=av, rs=rs, exs=exs, nkt=nkt,
                                nfull=nfull, nrs=nrs, rsi=rsi):
                        ex, lo = exs.pop(kt)
                        nc.tensor.matmul(
                            av[:, lo:],
                            v_sb[:, kt, h * 128:(h + 1) * 128],
                            ex[:, lo:],
                            start=(kt == 0), stop=(kt == nkt - 1),
                        )
                        if kt < nfull:
                            if kt % 2 == 0:
                                return
                            ex, lo = exs.pop(('p', kt)), 0
                        nc.tensor.matmul(
                            rs[:, lo:],
                            ones_sb[:],
                            ex[:, lo:],
                            start=(rsi[0] == 0), stop=(rsi[0] == nrs - 1),
                        )
                        rsi[0] += 1

                    for kt in range(min(3, nkt)):
                        produce(kt)
                    for kt in range(nkt):
                        if kt + 3 < nkt:
                            produce(kt + 3)
                        if kt == 2 and pend:
                            # previous chunk's normalization: its reciprocal
                            # has long finished on DVE
                            pend.pop(0)()
                        consume(kt)

                    # reciprocal right away (runs on DVE in the shadow of the
                    # next chunk's matmuls); broadcast + multiply deferred
                    rcb = tmp.tile([1, 512], bf16, tag="rcb", bufs=2)
                    rcp = tmp.tile([1, 512], f32, tag="rcp", bufs=1)
                    nc.vector.reciprocal_approx_fast(rcp[:], rs[:])
                    nc.vector.tensor_scalar_mul(rcb[:], rcp[:], 1.0)

                    def norm(h=h, c=c, av=av, rcb=rcb, attnT_sb=attnT_sb):
                        bc = psC.tile([128, 512], f32, tag="psC", name="bc")
                        nc.tensor.matmul(bc, onesr_sb[:], rcb[:])
                        bcs = tmp.tile([128, 512], bf16, tag="bcs", bufs=1)
                        nc.vector.tensor_scalar_mul(bcs[:], bc[:], 1.0)
                        nc.vector.tensor_mul(
                            attnT_sb[:, h, c * 512:(c + 1) * 512], av, bcs[:])

                    pend.append(norm)

                if b == 0:
                    load_wo_part(2 * h)
                    load_wo_part(2 * h + 1)

            while pend_rs:
                pend_rs.pop(0)()
            while pend:
                pend.pop(0)()
            stage_a2a(1)

        # -------- output projection for this core's row slices --------
        # core's out rows: [0:SL8] = batch-0 slice, [SL8:2*SL8] = batch-1 slice
        def proj_open(b, st, ec):
            # even heads: their A2A lands one collective earlier
            po = psB.tile([128, 512], f32, tag="psB", name="po")
            for r in range(n_cores):
                nc.tensor.matmul(
                    po,
                    atn_sb[b, 0][:, r, st * 128:(st + 1) * 128],
                    wo_sb[:, HSC * r, ec * 512:(ec + 1) * 512],
                    start=(r == 0), stop=False,
                )
            return po

        def proj_close(b, st, ec, po):
            for r in range(n_cores):
                nc.tensor.matmul(
                    po,
                    atn_sb[b, 1][:, r, st * 128:(st + 1) * 128],
                    wo_sb[:, HSC * r + 1, ec * 512:(ec + 1) * 512],
                    start=False, stop=(r == n_cores - 1),
                )
            ot = tmp.tile([128, 512], f32, tag="ot")
            nc.scalar.copy(ot[:], po[:])
            r0 = b * SL8 + st * 128
            nc.sync.dma_start(out[r0:r0 + 128, ec * 512:(ec + 1) * 512], ot[:])

        load_atn(1, 0)
        load_atn(1, 1, split=2)
        chunks = [(st, ec) for st in range(SL8 // 128) for ec in range(ECH)]
        for b in range(B):
            for st, ec in chunks:
                proj_close(b, st, ec, proj_open(b, st, ec))

    nc.finalize()
    return nc


# ---------------------------------------------------------------- host code
def make_tables(S):
    half = HD // 2
    inv_freq = (1.0 / (10000.0 ** (np.arange(half, dtype=np.float32) / half)))
    pos = np.arange(S, dtype=np.float32)
    freqs = pos[:, None] * inv_freq[None, :]          # [S, half]
    cos = np.cos(freqs).astype(np.float32)            # [S, half]
    sin = np.sin(freqs).astype(np.float32)
    cosT = np.concatenate([cos, cos], axis=1).T       # [HD, S]
    # sign-folded sin: rows 0..63 get -sin, rows 64..127 get +sin
    sinsT = np.concatenate([-sin, sin], axis=1).T     # [HD, S]
    return np.ascontiguousarray(cosT), np.ascontiguousarray(sinsT)


def prepare_in_maps(x, W_qkv, W_o, S, D):
    import ml_dtypes
    bf16 = ml_dtypes.bfloat16

    S2 = B * S
    xT = np.ascontiguousarray(
        x.reshape(S2, D).T.astype(np.float32)).astype(bf16)
    cosT, sinsT = make_tables(S)
    cosT, sinsT = cosT.astype(bf16), sinsT.astype(bf16)
    k = np.arange(128)[:, None]
    t = np.arange(128)[None, :]
    tri = (t >= k).astype(bf16)                       # [128, 128]
    ones = np.ones((128, 1), bf16)
    onesr = np.ones((1, 128), bf16)
    wo_bf16 = W_o.astype(bf16)

    qw = W_qkv[:, 0 * H * HD:1 * H * HD]
    kw = W_qkv[:, 1 * H * HD:2 * H * HD]
    vw = W_qkv[:, 2 * H * HD:3 * H * HD]

    in_maps = []
    for c in range(N_CORES):
        h0 = c * HEADS_PER_CORE
        cols = slice(h0 * HD, (h0 + HEADS_PER_CORE) * HD)
        wqk_c = np.ascontiguousarray(
            np.concatenate([qw[:, cols], kw[:, cols]], axis=1)).astype(bf16)
        wv_c = np.ascontiguousarray(vw[:, cols]).astype(bf16)
        in_maps.append({
            "xt": xT, "wqk": wqk_c, "wv": wv_c, "wo": wo_bf16,
            "cost": cosT, "sins": sinsT, "trit": tri,
            "ones": ones, "onesr": onesr,
        })
    return in_maps


_NC_CACHE = {}


def run(x, W_qkv, W_o, S, D, trace=False, trace_kwargs=None):
    from concourse.bass_utils import run_bass_kernel_spmd

    key = (S, D)
    if key not in _NC_CACHE:
        _NC_CACHE[key] = build_nc(S=S, D=D)
    nc = _NC_CACHE[key]
    in_maps = prepare_in_maps(x, W_qkv, W_o, S, D)
    res = run_bass_kernel_spmd(
        nc, in_maps, core_ids=list(range(N_CORES)),
        trace=trace, **(trace_kwargs or {}),
    )
    SL8 = S // N_CORES
    full = np.empty((B, S, D), np.float32)
    for c in range(N_CORES):
        o = res.results[c]["out"]
        full[0, c * SL8:(c + 1) * SL8] = o[:SL8]
        full[1, c * SL8:(c + 1) * SL8] = o[SL8:]
    return full, res


def kernel(x, W_qkv, W_o):
    x = np.asarray(x)
    W_qkv = np.asarray(W_qkv)
    W_o = np.asarray(W_o)
    S, D = x.shape[1], x.shape[2]
    out, _ = run(x, W_qkv, W_o, S, D, trace=False)
    return out.astype(np.float32)
